# revision 75
# baseline (speedup 1.0000x reference)
"""Mixtral sparse-MoE block with per-expert LoRA adapters on 8 Trainium2 cores.

Problem shapes: B=2, S=1024, H=2048, F=7168, E=8, R=32, top-K=2.
T = B*S = 2048 tokens.

Sharding: tensor-parallel over the FFN dim F. Core c owns rows
[c*896:(c+1)*896] of W1/W3 (and the matching B1/B3 LoRA rows) and the same
columns of W2/A2. Everything after the silu is linear in
x2s = silu(x1)*x3*rw, so each core emits an exact partial [H, T] output over
its F-shard and the host sums the 8 partials.

Work split (device vs host):
- Host: gating (softmax + top-2), the tiny per-expert LoRA down-projections
  a1/a3 = x @ A{1,3}T masked per slot (m1/m3 uploads), the m2 expert masking,
  and the final LoRA up-projection lora2 = B2 @ mask*sum_cores(m2).
- Device (per core): everything dense per 512-token block, as one f-sweep:
  for each 128-row f-chunk, base1/base3 accumulate into a PSUM bank
  (16 bf16 matmuls each), then the per-slot LoRA up-projection is FOLDED
  into the same bank with a single fp8 DoubleRow matmul (contraction
  ER=256 in one instruction), so base+lora never materializes in SBUF and
  there is no vector add. Slot 1 reuses the bank via an exact fp8 delta
  mask (md = fp8(m1[slot1]) - fp8(m1[slot0]); supports are disjoint so the
  difference is exactly representable). Scale bookkeeping: W1/W3 carry
  x256, the fp8 lora factors carry x16 each; the silu and the x3*rw ops
  consume the PSUM with a fused /256.
- After the sweep: a2 = A2[shard] @ x2s per slot (raw, host masks), and
  down = W2[shard] @ (x2s_0 + x2s_1) on the slot-summed activations.

Layout is feature-major ([feature, token]) end to end. Matmuls: base/down/a2
operands bf16 (206-220 ns/MM at N=512, LDWEIGHTS fully hidden when the PE
queue stays deep); lora up-projections fp8e4m3 DoubleRow (~250 ns for 256
contraction - 1.7x). All PSUM tiles share one 8-slot rotation (every
phase draws from the full bank pool), and the down-proj outputs stage
through a 10-deep SBUF pool so PSUM-bank release never waits on the
HWDGE descriptor queue.
"""

import sys
from contextlib import ExitStack

import numpy as np

try:
    import concourse.bass as bass  # noqa: F401
except ImportError:
    sys.path.insert(0, "/opt/trn_rl_repo")

import ml_dtypes

import concourse.bass as bass
import concourse.mybir as mybir
import concourse.tile as tile
from concourse import bacc
from concourse.bass_utils import run_bass_kernel_spmd

BF16 = mybir.dt.bfloat16
FP8 = mybir.dt.float8e4
F32 = mybir.dt.float32
NPBF16 = ml_dtypes.bfloat16
NPFP8 = ml_dtypes.float8_e4m3

B, S, H, F, E, R, K = 2, 1024, 2048, 7168, 8, 32, 2
T = B * S                      # 2048 tokens
ER = E * R                     # 256
NCORES = 8
FS = F // NCORES               # 896 per-core F shard
NH = H // 128                  # 16 h-chunks
NF = FS // 128                 # 7 f-chunks (per core)
TBLK = 512
NT = T // TBLK                 # 4 token blocks

SW = 256.0                     # base scale carried by W1/W3
SL = 16.0                      # fp8 lora scale per factor (16*16 = SW)
INV_SW = 1.0 / SW
SL2 = 8.0                      # fp8 scale on the x2 shadow for the a2 GEMM
A2S = 16.0                     # fp8 scale on the a2 stationary (SL2*A2S=128)

DR = mybir.MatmulPerfMode.DoubleRow
MULT = mybir.AluOpType.mult


def build_nc(repeat=None, phases="ABZC"):
    # phases: A = base matmuls, B = lora/silu/x2 chain, Z = a2, C = down-proj
    nc = bacc.Bacc(None)

    xT = nc.declare_dram_parameter("xT", [NH, 128, T], BF16, isOutput=False)
    w1t = nc.declare_dram_parameter("w1t", [NH, 128, FS], BF16, isOutput=False)
    w3t = nc.declare_dram_parameter("w3t", [NH, 128, FS], BF16, isOutput=False)
    w2t = nc.declare_dram_parameter("w2t", [NF, 128, H], BF16, isOutput=False)
    # per-slot fp8 lora factors in DoubleRow layout: index 0 = slot-0 mask,
    # index 1 = exact fp8 delta (slot1 - slot0)
    m1t = nc.declare_dram_parameter("m1t", [K, 128, 2, T], FP8, isOutput=False)
    m3t = nc.declare_dram_parameter("m3t", [K, 128, 2, T], FP8, isOutput=False)
    b1t = nc.declare_dram_parameter("b1t", [128, 2, FS], FP8, isOutput=False)
    b3t = nc.declare_dram_parameter("b3t", [128, 2, FS], FP8, isOutput=False)
    # a2 stationary: 3 DoubleRow f-pairs (fp8, x16) + bf16 leftover f=6 (x128)
    a2t = nc.declare_dram_parameter("a2t", [NF // 2, 128, 2, ER], FP8,
                                    isOutput=False)
    a2l = nc.declare_dram_parameter("a2l", [128, ER], BF16, isOutput=False)
    rwr = nc.declare_dram_parameter("rwr", [K, 1, T], BF16, isOutput=False)
    outT = nc.declare_dram_parameter("outT", [NH, 128, T], BF16, isOutput=True)
    m2o = nc.declare_dram_parameter("m2o", [K, ER // 128, 128, T], BF16,
                                    isOutput=True)

    with tile.TileContext(nc) as tc, ExitStack() as ctx:
        resw = ctx.enter_context(tc.tile_pool(name="resw", bufs=1))
        xsp = ctx.enter_context(tc.tile_pool(name="xsp", bufs=2))
        actp = ctx.enter_context(tc.tile_pool(name="actp", bufs=1))
        mp_ = ctx.enter_context(tc.tile_pool(name="mp", bufs=2))
        trans = ctx.enter_context(tc.tile_pool(name="trans", bufs=3))
        outp = ctx.enter_context(tc.tile_pool(name="outp", bufs=4))
        # deep staging for the down-proj outputs: the HWDGE descriptor
        # device is near-saturated during the a2/C window (next-block
        # prefetch + m2o + outT), so copies must be able to run far ahead
        # of the DMA drain or they hold the pD PSUM banks and stall the PE
        osbp = ctx.enter_context(tc.tile_pool(name="osbp", bufs=10))
        psp = ctx.enter_context(tc.tile_pool(name="psp", bufs=8, space="PSUM"))

        if repeat is not None:
            ctx.enter_context(tc.For_i(0, repeat))

        def load_block_inputs(tb, xs=None):
            tsl = slice(tb * TBLK, (tb + 1) * TBLK)
            if xs is None:
                xs = []
                for h in range(NH):
                    xt_ = xsp.tile([128, TBLK], BF16, name=f"x{h}", tag=f"x{h}")
                    nc.sync.dma_start(out=xt_, in_=xT[h][:, tsl])
                    xs.append(xt_)
            m1 = []
            m3 = []
            for j in range(K if "B" in phases else 0):
                m1_ = mp_.tile([128, 2, TBLK], FP8, name=f"m1_{j}", tag=f"m1_{j}")
                nc.sync.dma_start(out=m1_, in_=m1t[j][:, :, tsl])
                m1.append(m1_)
                m3_ = mp_.tile([128, 2, TBLK], FP8, name=f"m3_{j}", tag=f"m3_{j}")
                nc.sync.dma_start(out=m3_, in_=m3t[j][:, :, tsl])
                m3.append(m3_)
            rws = []
            for k in range(K if "B" in phases else 0):
                r_ = mp_.tile([128, TBLK], BF16, name=f"rw{k}", tag=f"rw{k}")
                nc.sync.dma_start(out=r_, in_=rwr[k][:, tsl].to_broadcast([128, TBLK]))
                rws.append(r_)
            return xs, m1, m3, rws

        # ---- resident weights, emitted in first-use order ----
        xs0 = []
        w1s, w3s = [], []
        for h in range(NH):
            xt_ = xsp.tile([128, TBLK], BF16, name=f"x{h}", tag=f"x{h}")
            nc.sync.dma_start(out=xt_, in_=xT[h][:, 0:TBLK])
            xs0.append(xt_)
            t1 = resw.tile([128, FS], BF16, name=f"w1s{h}", tag=f"w1s{h}")
            nc.sync.dma_start(out=t1, in_=w1t[h])
            w1s.append(t1)
            t3 = resw.tile([128, FS], BF16, name=f"w3s{h}", tag=f"w3s{h}")
            nc.sync.dma_start(out=t3, in_=w3t[h])
            w3s.append(t3)
        if "B" in phases:
            b1dr = resw.tile([128, 2, FS], FP8, name="b1dr", tag="b1dr")
            nc.sync.dma_start(out=b1dr, in_=b1t[:, :, :])
            b3dr = resw.tile([128, 2, FS], FP8, name="b3dr", tag="b3dr")
            nc.sync.dma_start(out=b3dr, in_=b3t[:, :, :])
        pre0 = load_block_inputs(0, xs0)
        w2s = []
        for f in range(NF if "C" in phases else 0):
            t_ = resw.tile([128, H], BF16, name=f"w2s{f}", tag=f"w2s{f}")
            nc.sync.dma_start(out=t_, in_=w2t[f])
            w2s.append(t_)
        a2s = []
        a2last = None
        if "Z" in phases:
            for p in range(NF // 2):
                t_ = resw.tile([128, 2, ER], FP8, name=f"a2s{p}", tag=f"a2s{p}")
                nc.sync.dma_start(out=t_, in_=a2t[p])
                a2s.append(t_)
            a2last = resw.tile([128, ER], BF16, name="a2l", tag="a2l")
            nc.sync.dma_start(out=a2last, in_=a2l[:, :])

        pending = pre0
        xsums = [[None] * NF for _ in range(NT)]
        for tb in range(NT):
            tsl = slice(tb * TBLK, (tb + 1) * TBLK)
            xs, m1, m3, rws = pending

            # ---- f-sweep: base accumulation + folded lora + silu chain ----
            x2s = [[None] * NF for _ in range(K)]
            x2q = [[actp.tile([128, 2, TBLK], FP8, name=f"x2q_{k}{p}",
                              tag=f"x2q_{k}{p}")
                    for p in range(NF // 2)] for k in range(K)] \
                if "Z" in phases else None
            xsum = xsums[tb]

            def emit_slot(k, f, ps1, ps3):
                fsl = slice(f * 128, (f + 1) * 128)
                nc.tensor.matmul(ps1, b1dr[:, :, fsl], m1[k],
                                 start=False, stop=(k == K - 1),
                                 perf_mode=DR, skip_group_check=(k > 0))
                nc.tensor.matmul(ps3, b3dr[:, :, fsl], m3[k],
                                 start=False, stop=(k == K - 1),
                                 perf_mode=DR, skip_group_check=(k > 0))
                sl_ = trans.tile([128, TBLK], BF16, name=f"sl{k}",
                                 tag=f"sl{k}")
                nc.scalar.activation(sl_, ps1,
                                     mybir.ActivationFunctionType.Silu,
                                     scale=INV_SW)
                x3_ = trans.tile([128, TBLK], BF16, name=f"x3s{k}",
                                 tag=f"x3s{k}")
                nc.vector.scalar_tensor_tensor(x3_, ps3, INV_SW, rws[k],
                                               MULT, MULT)
                x2_ = actp.tile([128, TBLK], BF16, name=f"x2_{k}{f}",
                                tag=f"x2_{k}{f}")
                nc.vector.tensor_mul(x2_, sl_, x3_)
                x2s[k][f] = x2_
                if "Z" in phases and f < NF - 1:
                    # fp8 shadow (x8) in DoubleRow pair layout for the a2 GEMM
                    q_ = x2q[k][f // 2]
                    nc.vector.tensor_scalar_mul(q_[:, f % 2, :], x2_, SL2)
                if k == K - 1:
                    xs_ = actp.tile([128, TBLK], BF16, name=f"xsum{tb}_{f}",
                                    tag=f"xsum{tb}_{f}")
                    nc.vector.tensor_add(xs_, x2s[0][f], x2s[1][f])
                    xsum[f] = xs_

            def emit_base(f, ps1, ps3, h0, h1):
                fsl = slice(f * 128, (f + 1) * 128)
                for h in range(h0, h1):
                    nc.tensor.matmul(ps1, w1s[h][:, fsl], xs[h],
                                     start=(h == 0), stop=False)
                    nc.tensor.matmul(ps3, w3s[h][:, fsl], xs[h],
                                     start=(h == 0), stop=False)

            for f in range(NF):
                ps1 = psp.tile([128, TBLK], F32, name="ps1", tag="ps")
                ps3 = psp.tile([128, TBLK], F32, name="ps3", tag="ps")
                if "A" in phases:
                    emit_base(f, ps1, ps3, 0, NH)
                if "B" not in phases:
                    continue
                emit_slot(0, f, ps1, ps3)
                emit_slot(1, f, ps1, ps3)

            # prefetch next block's streamed inputs during a2/down
            if tb + 1 < NT:
                pending = load_block_inputs(tb + 1)

            # ---- a2: raw per-slot LoRA down-projection (host masks).
            # 3 fp8 DoubleRow f-pairs + one bf16 leftover; PSUM at x128. ----
            for k in range(K if "Z" in phases else 0):
                for er in range(ER // 128):
                    ers = slice(er * 128, (er + 1) * 128)
                    psa2 = psp.tile([128, TBLK], F32, name="psa2", tag="ps")
                    for p in range(NF // 2):
                        nc.tensor.matmul(psa2, a2s[p][:, :, ers], x2q[k][p],
                                         start=(p == 0), stop=False,
                                         perf_mode=DR)
                    nc.tensor.matmul(psa2, a2last[:, ers], x2s[k][NF - 1],
                                     start=False, stop=True)
                    m2_ = outp.tile([128, TBLK], BF16, name=f"m2_{k}{er}",
                                    tag="m2sb")
                    nc.vector.tensor_copy(m2_, psa2)
                    nc.sync.dma_start(out=m2o[k][er][:, tsl], in_=m2_)

            # ---- down-proj on the slot-summed activations ----
            for h in range(NH if "C" in phases else 0):
                hsl = slice(h * 128, (h + 1) * 128)
                psD = psp.tile([128, TBLK], F32, name="psD", tag="ps")
                for f in range(NF):
                    nc.tensor.matmul(psD, w2s[f][:, hsl], xsum[f],
                                     start=(f == 0), stop=(f == NF - 1))
                o_ = osbp.tile([128, TBLK], BF16, name="osb", tag="osb")
                nc.vector.tensor_copy(o_, psD)
                nc.sync.dma_start(out=outT[h][:, tsl], in_=o_)

    nc.finalize()
    return nc


def _to_dr(a2d):
    """[256, N] -> DoubleRow fp8 layout [128, 2, N]."""
    return np.ascontiguousarray(
        a2d.reshape(2, 128, -1).transpose(1, 0, 2))


def prepare_inputs(hidden_states, Wg, W1, W2, W3, A1, B1, A2, B2, A3, B3):
    """Host preprocessing: routing + per-core weight slicing/casting."""
    hidden_states, Wg, W1, W2, W3, A1, B1, A2, B2, A3, B3 = (
        np.asarray(a, dtype=np.float32)
        for a in (hidden_states, Wg, W1, W2, W3, A1, B1, A2, B2, A3, B3))
    x = np.ascontiguousarray(hidden_states.reshape(T, H))

    logits = x @ Wg.T.astype(np.float32)
    m = logits.max(-1, keepdims=True)
    p = np.exp(logits - m, dtype=np.float32)
    p /= p.sum(-1, keepdims=True)
    sel = np.argsort(-p, axis=-1, kind="stable")[:, :K]      # [T, K]
    rw = np.take_along_axis(p, sel, axis=1)
    rw = (rw / rw.sum(-1, keepdims=True)).astype(np.float32)  # [T, K]

    xT_np = np.ascontiguousarray(x.T).astype(NPBF16).reshape(NH, 128, T)
    rwr_np = np.ascontiguousarray(rw.T).reshape(K, 1, T).astype(NPBF16)

    # per-slot one-hot masks over the (e, r) axis, [K, ER, T]; device gets
    # the fp8-quantized slot-0 factors and the exact fp8 slot delta
    masks = np.zeros((K, ER, T), dtype=np.float32)
    for k in range(K):
        onehot = np.zeros((T, E), np.float32)
        onehot[np.arange(T), sel[:, k]] = 1.0
        masks[k] = np.repeat(onehot, R, axis=1).T

    A1f = A1.reshape(ER, H)                      # [er, H]
    A3f = A3.reshape(ER, H)
    B2f = B2.transpose(0, 2, 1).reshape(ER, H)   # [er, H]

    a1_all = x @ A1f.T.astype(np.float32)        # [T, ER]
    a3_all = x @ A3f.T.astype(np.float32)

    def slot_factors(a_all):
        q = [np.asarray((a_all.T * masks[k] * SL).astype(NPFP8), np.float32)
             for k in range(K)]                  # [ER, T] quantized, scaled
        d = q[1] - q[0]                          # exact in fp8 (disjoint)
        return (_to_dr(q[0].astype(NPFP8)).reshape(1, 128, 2, T),
                _to_dr(d.astype(NPFP8)).reshape(1, 128, 2, T))

    m1q0, m1d = slot_factors(a1_all)
    m3q0, m3d = slot_factors(a3_all)
    m1t_np = np.concatenate([m1q0, m1d], axis=0)   # [K, 128, 2, T]
    m3t_np = np.concatenate([m3q0, m3d], axis=0)

    in_maps = []
    for c in range(NCORES):
        fs = slice(c * FS, (c + 1) * FS)
        w1T = np.ascontiguousarray(W1[fs].T * SW).astype(NPBF16)   # [H, FS]
        w3T = np.ascontiguousarray(W3[fs].T * SW).astype(NPBF16)
        w1t_np = w1T.reshape(NH, 128, FS)
        w3t_np = w3T.reshape(NH, 128, FS)
        w2T = np.ascontiguousarray(W2[:, fs].T).astype(NPBF16)  # [FS, H]
        w2t_np = w2T.reshape(NF, 128, H)
        b1f = B1[:, fs, :].transpose(0, 2, 1).reshape(ER, FS) * SL  # [er, f]
        b3f = B3[:, fs, :].transpose(0, 2, 1).reshape(ER, FS) * SL
        b1t_np = _to_dr(b1f.astype(NPFP8))
        b3t_np = _to_dr(b3f.astype(NPFP8))
        a2f = A2[:, :, fs].reshape(ER, FS)                      # [er, f]
        r = np.ascontiguousarray(a2f.T).reshape(NF, 128, ER)    # [fchunk,ki,er]
        a2t_np = np.ascontiguousarray(
            np.stack([r[0:6:2], r[1:6:2]], axis=2) * A2S).astype(NPFP8)
        a2l_np = np.ascontiguousarray(r[6] * SL2 * A2S).astype(NPBF16)

        in_maps.append({
            "xT": xT_np, "w1t": w1t_np, "w3t": w3t_np, "w2t": w2t_np,
            "m1t": m1t_np, "m3t": m3t_np, "b1t": b1t_np, "b3t": b3t_np,
            "a2t": a2t_np, "a2l": a2l_np, "rwr": rwr_np,
        })
    return in_maps, (B2f.astype(np.float32), masks)


_CACHED_NC = None


def kernel(hidden_states, Wg, W1, W2, W3, A1, B1, A2, B2, A3, B3,
           _trace=False, _tmpdir=None):
    global _CACHED_NC
    in_maps, (B2f, masks) = prepare_inputs(hidden_states, Wg, W1, W2, W3,
                                           A1, B1, A2, B2, A3, B3)
    if _CACHED_NC is None:
        _CACHED_NC = build_nc()
    nc = _CACHED_NC
    res = run_bass_kernel_spmd(nc, in_maps, list(range(NCORES)),
                               trace=_trace, tmpdir=_tmpdir)
    acc = np.zeros((NH, 128, T), np.float32)
    m2sum = np.zeros((K, ER, T), np.float32)
    for c in range(NCORES):
        acc += res.results[c]["outT"].astype(np.float32)
        m2sum += res.results[c]["m2o"].reshape(K, ER, T).astype(np.float32)
    out = acc.reshape(H, T)
    m2sum *= 1.0 / (SL2 * A2S)     # device a2 runs at x128 scale
    # host-side lora2: mask the raw per-slot a2 sums, then one small GEMM
    for k in range(K):
        out += B2f.T @ (m2sum[k] * masks[k])
    out = out.T.reshape(B, S, H)
    kernel.last_results = res
    return out


if __name__ == "__main__":
    nc = build_nc()
    print("built ok")
